# revision 8
# baseline (speedup 1.0000x reference)
"""Trainium2 Bass kernel for the ReLU-RNN problem (nn_RNN).

Math (per core, data-parallel over batch; 8 batch rows per core):
    xp  = x @ W_ih.T + b_ih + b_hh          (big GEMM, phase 1)
    h_t = relu(xp_t + h_{t-1} @ W_hh.T)     (S sequential steps, phase 2)

All recurrent state lives transposed: h^T is [H=512 (4 partition
chunks), B_local=8].  Each step is 16 bf16 matmuls (stationary =
128x128 tiles of W_hh^T, moving = h^T chunks [128, 8]) accumulated in
fp32 PSUM, then add-xp + relu split per output h-chunk so consecutive
steps pipeline on the PE without stalls (m-outer / k-outer orderings
alternate between steps).  Every 16 steps the h^T history window is
PE-transposed back to batch-major and DMAed out as [b, t, :] rows.
"""

import numpy as np

import concourse.bass as bass
import concourse.mybir as mybir
from concourse import masks, tile
from concourse.bass_utils import run_bass_kernel_spmd

B, S, I, H = 64, 1024, 512, 512
NCORES = 8
BL = B // NCORES          # batch rows per core
KT = I // 128             # contraction tiles (4)
MT = H // 128             # output-row tiles (4)
WIN = 16                  # recurrence steps per output window
F32 = mybir.dt.float32
BF16 = mybir.dt.bfloat16

# This walrus build rejects instructions carrying more than one sync-wait
# command.  After Tile scheduling, split any excess waits onto same-engine
# NOP instructions inserted immediately before the offending instruction.
_MAX_WAITS = 1


def _split_multi_waits(nc):
    cnt = 0
    for bb in nc.main_func.blocks:
        insts = bb.instructions
        i = 0
        while i < len(insts):
            inst = insts[i]
            si = getattr(inst, "sync_info", None)
            if si is not None and len(si.on_wait) > _MAX_WAITS:
                waits = list(si.on_wait)
                si.on_wait = waits[-_MAX_WAITS:]
                rest = waits[:-_MAX_WAITS]
                for j in range(0, len(rest), _MAX_WAITS):
                    nop = mybir.InstNoOp(
                        name=f"ws_{cnt}",
                        engine=inst.engine,
                        sync_info=mybir.SyncInfo(
                            on_wait=rest[j : j + _MAX_WAITS], on_update=[]
                        ),
                        bass_nofuse=True,
                    )
                    cnt += 1
                    insts.insert(i, nop)
                    i += 1
            i += 1
    return cnt


def build_rnn_kernel(seq_len=S):
    SL = seq_len
    NW = SL // WIN
    assert SL % WIN == 0
    nc = bass.Bass(target_bir_lowering=False, trn_type="TRN2")

    x_d = nc.dram_tensor("inputs", [BL, SL, I], F32, kind="ExternalInput")
    h0_d = nc.dram_tensor("h0", [BL, H], F32, kind="ExternalInput")
    wih_d = nc.dram_tensor("weight_ih", [H, I], F32, kind="ExternalInput")
    whh_d = nc.dram_tensor("weight_hh", [H, H], F32, kind="ExternalInput")
    bih_d = nc.dram_tensor("bias_ih", [H], F32, kind="ExternalInput")
    bhh_d = nc.dram_tensor("bias_hh", [H], F32, kind="ExternalInput")
    out_d = nc.dram_tensor("outputs", [BL, SL, H], F32, kind="ExternalOutput")
    hf_d = nc.dram_tensor("h_final", [BL, H], F32, kind="ExternalOutput")

    with tile.TileContext(nc) as tc:
        with tc.tile_pool(name="const", bufs=1) as constp:
            ident = constp.tile([128, 128], BF16)
            masks.make_identity(nc, ident[:])

            # bias_t[p, hc] = (b_ih + b_hh)[hc*128 + p]
            bias_t = constp.tile([128, MT], F32)
            btmp = constp.tile([128, MT], F32)
            nc.sync.dma_start(bias_t[:], bih_d.rearrange("(hc p) -> p hc", p=128))
            nc.sync.dma_start(btmp[:], bhh_d.rearrange("(hc p) -> p hc", p=128))
            nc.vector.tensor_add(bias_t[:], bias_t[:], btmp[:])

            # lhsT weight tiles: lhsT(kc, mc)[k, m] = W[mc*128+m, kc*128+k]
            wih_T = constp.tile([128, KT * MT * 128], BF16)   # col idx (kc*MT+mc)
            whh_T = constp.tile([128, KT * MT * 128], BF16)
            # xp^T, SBUF-resident, col idx (t, hc, b)
            xp_t = constp.tile([128, SL * MT * BL], F32)
            # initial h^T staging window (holds h0 in slot WIN-1)
            stag_init = constp.tile([128, WIN * MT * BL], BF16)

            with (
                tc.tile_pool(name="wld", bufs=2) as wldp,
                tc.tile_pool(name="wtp", bufs=2, space="PSUM") as wtpp,
            ):
                for w_d, w_T in ((wih_d, wih_T), (whh_d, whh_T)):
                    for mc in range(MT):
                        wld = wldp.tile([128, 512], F32, tag="wld")
                        wldb = wldp.tile([128, 512], BF16, tag="wldb")
                        nc.sync.dma_start(wld[:], w_d[mc * 128 : (mc + 1) * 128, :])
                        nc.vector.tensor_copy(wldb[:], wld[:])
                        for kc in range(KT):
                            ps = wtpp.tile([128, 128], BF16, tag="wtp")
                            nc.tensor.transpose(
                                ps[:], wldb[:, kc * 128 : (kc + 1) * 128], ident[:]
                            )
                            j = (kc * MT + mc) * 128
                            nc.vector.tensor_copy(w_T[:, j : j + 128], ps[:])

                # h0 -> bf16, transposed into stag_init slot WIN-1
                h0l = wldp.tile([BL, H], F32, tag="h0l")
                h0b = wldp.tile([BL, H], BF16, tag="h0b")
                nc.sync.dma_start(h0l[:], h0_d[:, :])
                nc.vector.tensor_copy(h0b[:], h0l[:])
                for hc in range(MT):
                    ps = wtpp.tile([128, 128], BF16, tag="wtp")
                    nc.tensor.transpose(
                        ps[:, :BL], h0b[:, hc * 128 : (hc + 1) * 128],
                        ident[:BL, :BL],
                    )
                    off = hc * WIN * BL + (WIN - 1) * BL
                    nc.vector.tensor_copy(
                        stag_init[:, off : off + BL], ps[:, :BL]
                    )

            # view of xp^T as [p, t, hc, b]
            xp_v = xp_t[:].rearrange("p (t hc b) -> p t hc b", hc=MT, b=BL)

            # ========== phase 1: xp^T = W_ih @ x^T + bias ==========
            TC = 512  # timesteps per block (one batch row at a time)
            with (
                tc.tile_pool(name="xld", bufs=3) as xldp,
                tc.tile_pool(name="xtr", bufs=2) as xtrp,
                tc.tile_pool(name="xps", bufs=4, space="PSUM") as xpsp,
                tc.tile_pool(name="mmp", bufs=2, space="PSUM") as mmpp,
            ):
                for b in range(BL):
                    for tc0 in range(0, SL, TC):
                        ntc = min(TC, SL - tc0)
                        xT = [
                            xtrp.tile(
                                [128, TC], BF16, tag=f"xT{ic}", name=f"xT{ic}"
                            )
                            for ic in range(KT)
                        ]
                        for tb in range((ntc + 127) // 128):
                            nb = min(128, ntc - tb * 128)
                            xld = xldp.tile([128, 512], F32, tag="xld")
                            xlb = xldp.tile([128, 512], BF16, tag="xlb")
                            r0 = tc0 + tb * 128
                            nc.sync.dma_start(
                                xld[:nb, :], x_d[b, r0 : r0 + nb, :]
                            )
                            nc.vector.tensor_copy(xlb[:nb, :], xld[:nb, :])
                            for ic in range(KT):
                                ps = xpsp.tile([128, 128], BF16, tag="xtp")
                                nc.tensor.transpose(
                                    ps[:, :nb],
                                    xlb[:nb, ic * 128 : (ic + 1) * 128],
                                    ident[:nb, :nb],
                                )
                                nc.vector.tensor_copy(
                                    xT[ic][:, tb * 128 : tb * 128 + nb],
                                    ps[:, :nb],
                                )
                        for hc in range(MT):
                            ps = mmpp.tile([128, TC], F32, tag="xpmm")
                            for ic in range(KT):
                                j = (ic * MT + hc) * 128
                                nc.tensor.matmul(
                                    ps[:, :ntc],
                                    wih_T[:, j : j + 128],
                                    xT[ic][:, :ntc],
                                    start=(ic == 0),
                                    stop=(ic == KT - 1),
                                )
                            # xp_v[:, tc0:tc0+ntc, hc, b] += bias
                            nc.scalar.add(
                                xp_v[:, tc0 : tc0 + ntc, hc, b],
                                ps[:, :ntc],
                                bias_t[:, hc : hc + 1],
                            )

            # ========== phase 2: the recurrence ==========
            with (
                tc.tile_pool(name="rec", bufs=2, space="PSUM") as recp,
                tc.tile_pool(name="tmp", bufs=2) as tmpp,
                tc.tile_pool(name="stg", bufs=3) as stgp,
                tc.tile_pool(name="otr", bufs=2, space="PSUM") as otrp,
                tc.tile_pool(name="owd", bufs=3) as owdp,
            ):
                def finalize_window(w, stag_w):
                    """Transpose window w to batch-major and DMA it out."""
                    ow = owdp.tile([128, H], F32, tag="ow")
                    for hc in range(MT):
                        ps = otrp.tile([128, 128], BF16, tag="otr")
                        src_ap = stag_w[:, hc * WIN * BL : (hc + 1) * WIN * BL]
                        nc.tensor.transpose(ps[:], src_ap, ident[:])
                        nc.vector.tensor_copy(
                            ow[:, hc * 128 : (hc + 1) * 128], ps[:]
                        )
                    dst = out_d[:, w * WIN : (w + 1) * WIN, :].rearrange(
                        "b t h -> t b h"
                    )
                    nc.sync.dma_start(dst, ow[:])
                    return ow

                stag_prev = stag_init
                stag_cur = None
                for t in range(SL):
                    tl = t % WIN
                    if tl == 0:
                        if t > 0:
                            finalize_window(t // WIN - 1, stag_prev)
                        stag_cur = stgp.tile([128, WIN * MT * BL], BF16, tag="stg")
                    ps = recp.tile([128, MT * BL], F32, tag="rec")
                    tmp = tmpp.tile([128, MT * BL], F32, tag="tmp")
                    ptl = (WIN - 1) if tl == 0 else (tl - 1)
                    hprev = stag_prev if tl == 0 else stag_cur
                    sc = stag_cur

                    def mm(k, m, first):
                        j = (k * MT + m) * 128
                        o = k * WIN * BL + ptl * BL
                        # start only on the step's first matmul: start=True
                        # marks the WHOLE 2KB bank pending-zero, so later
                        # groups in the same bank would otherwise discard
                        # their earlier accumulations.
                        nc.tensor.matmul(
                            ps[:, m * BL : (m + 1) * BL],
                            whh_T[:, j : j + 128],
                            hprev[:, o : o + BL],
                            start=first,
                            stop=(k == KT - 1),
                            skip_group_check=True,
                        )

                    def epilogue(m):
                        sl = slice(m * BL, (m + 1) * BL)
                        nc.vector.tensor_add(
                            tmp[:, sl], ps[:, sl], xp_v[:, t, m, :]
                        )
                        o = m * WIN * BL + tl * BL
                        nc.scalar.activation(
                            sc[:, o : o + BL],
                            tmp[:, sl],
                            mybir.ActivationFunctionType.Relu,
                        )

                    if t % 2 == 0:  # m-outer: m-groups complete early->late
                        for m in range(MT):
                            for k in range(KT):
                                mm(k, m, first=(m == 0 and k == 0))
                            epilogue(m)
                    else:  # k-outer: consumes h chunks in k order
                        for k in range(KT):
                            for m in range(MT):
                                mm(k, m, first=(m == 0 and k == 0))
                        for m in range(MT):
                            epilogue(m)
                    if tl == WIN - 1:
                        stag_prev = stag_cur

                last_ow = finalize_window(NW - 1, stag_prev)
                # h_final: rows t=WIN-1 of the last window = partitions 120..127
                nc.sync.dma_start(
                    hf_d[:, :], last_ow[(WIN - 1) * BL : WIN * BL, :]
                )

    _split_multi_waits(nc)
    return nc


_NC_CACHE = {}


def get_nc(seq_len=S):
    if seq_len not in _NC_CACHE:
        _NC_CACHE[seq_len] = build_rnn_kernel(seq_len)
    return _NC_CACHE[seq_len]


def make_in_maps(inputs, h0, weight_ih, weight_hh, bias_ih, bias_hh):
    inputs = np.ascontiguousarray(np.asarray(inputs, dtype=np.float32))
    h0 = np.asarray(h0, dtype=np.float32).reshape(-1, H)
    weight_ih = np.ascontiguousarray(np.asarray(weight_ih, dtype=np.float32))
    weight_hh = np.ascontiguousarray(np.asarray(weight_hh, dtype=np.float32))
    bias_ih = np.ascontiguousarray(np.asarray(bias_ih, dtype=np.float32))
    bias_hh = np.ascontiguousarray(np.asarray(bias_hh, dtype=np.float32))
    in_maps = []
    for c in range(NCORES):
        sl = slice(c * BL, (c + 1) * BL)
        in_maps.append(
            {
                "inputs": np.ascontiguousarray(inputs[sl]),
                "h0": np.ascontiguousarray(h0[sl]),
                "weight_ih": weight_ih,
                "weight_hh": weight_hh,
                "bias_ih": bias_ih,
                "bias_hh": bias_hh,
            }
        )
    return in_maps


def assemble(results, seq_len=S):
    outputs = np.empty((B, seq_len, H), dtype=np.float32)
    h_final = np.empty((B, H), dtype=np.float32)
    for c in range(NCORES):
        sl = slice(c * BL, (c + 1) * BL)
        outputs[sl] = results[c]["outputs"]
        h_final[sl] = results[c]["h_final"]
    return outputs, h_final[None, :, :]


def kernel(inputs, h0, weight_ih, weight_hh, bias_ih, bias_hh):
    nc = get_nc(S)
    in_maps = make_in_maps(inputs, h0, weight_ih, weight_hh, bias_ih, bias_hh)
    res = run_bass_kernel_spmd(nc, in_maps, core_ids=list(range(NCORES)))
    return assemble(res.results, S)


# revision 9
# speedup vs baseline: 1.3062x; 1.3062x over previous
"""Trainium2 Bass kernel for the ReLU-RNN problem (nn_RNN).

Math (per core, data-parallel over batch; 8 batch rows per core):
    xp  = x @ W_ih.T + b_ih + b_hh          (big GEMM, phase 1)
    h_t = relu(xp_t + h_{t-1} @ W_hh.T)     (S sequential steps, phase 2)

All recurrent state lives transposed: h^T is [H=512 (4 partition
chunks), B_local=8].  Each step is 16 bf16 matmuls (stationary =
128x128 tiles of W_hh^T, moving = h^T chunks [128, 8]) accumulated in
fp32 PSUM, then add-xp + relu split per output h-chunk so consecutive
steps pipeline on the PE without stalls (m-outer / k-outer orderings
alternate between steps).  Every 16 steps the h^T history window is
PE-transposed back to batch-major and DMAed out as [b, t, :] rows.
"""

import numpy as np

import concourse.bass as bass
import concourse.mybir as mybir
from concourse import masks, tile
from concourse.bass_utils import run_bass_kernel_spmd

B, S, I, H = 64, 1024, 512, 512
NCORES = 8
BL = B // NCORES          # batch rows per core
KT = I // 128             # contraction tiles (4)
MT = H // 128             # output-row tiles (4)
WIN = 16                  # recurrence steps per output window
F32 = mybir.dt.float32
BF16 = mybir.dt.bfloat16

# This walrus build rejects instructions carrying more than one sync-wait
# command.  After Tile scheduling, split any excess waits onto same-engine
# NOP instructions inserted immediately before the offending instruction.
_MAX_WAITS = 1


def _split_multi_waits(nc):
    cnt = 0
    for bb in nc.main_func.blocks:
        insts = bb.instructions
        i = 0
        while i < len(insts):
            inst = insts[i]
            si = getattr(inst, "sync_info", None)
            if si is not None and len(si.on_wait) > _MAX_WAITS:
                waits = list(si.on_wait)
                si.on_wait = waits[-_MAX_WAITS:]
                rest = waits[:-_MAX_WAITS]
                for j in range(0, len(rest), _MAX_WAITS):
                    nop = mybir.InstNoOp(
                        name=f"ws_{cnt}",
                        engine=inst.engine,
                        sync_info=mybir.SyncInfo(
                            on_wait=rest[j : j + _MAX_WAITS], on_update=[]
                        ),
                        bass_nofuse=True,
                    )
                    cnt += 1
                    insts.insert(i, nop)
                    i += 1
            i += 1
    return cnt


def build_rnn_kernel(seq_len=S):
    SL = seq_len
    NW = SL // WIN
    assert SL % WIN == 0
    nc = bass.Bass(target_bir_lowering=False, trn_type="TRN2")

    x_d = nc.dram_tensor("inputs", [BL, SL, I], F32, kind="ExternalInput")
    h0_d = nc.dram_tensor("h0", [BL, H], F32, kind="ExternalInput")
    wih_d = nc.dram_tensor("weight_ih", [H, I], F32, kind="ExternalInput")
    whh_d = nc.dram_tensor("weight_hh", [H, H], F32, kind="ExternalInput")
    bih_d = nc.dram_tensor("bias_ih", [H], F32, kind="ExternalInput")
    bhh_d = nc.dram_tensor("bias_hh", [H], F32, kind="ExternalInput")
    out_d = nc.dram_tensor("outputs", [BL, SL, H], F32, kind="ExternalOutput")
    hf_d = nc.dram_tensor("h_final", [BL, H], F32, kind="ExternalOutput")

    with tile.TileContext(nc) as tc:
        with tc.tile_pool(name="const", bufs=1) as constp:
            ident = constp.tile([128, 128], BF16)
            masks.make_identity(nc, ident[:])

            # bias_t[p, hc] = (b_ih + b_hh)[hc*128 + p]
            bias_t = constp.tile([128, MT], F32)
            btmp = constp.tile([128, MT], F32)
            nc.sync.dma_start(bias_t[:], bih_d.rearrange("(hc p) -> p hc", p=128))
            nc.sync.dma_start(btmp[:], bhh_d.rearrange("(hc p) -> p hc", p=128))
            nc.vector.tensor_add(bias_t[:], bias_t[:], btmp[:])

            # lhsT weight tiles: lhsT(kc, mc)[k, m] = W[mc*128+m, kc*128+k]
            wih_T = constp.tile([128, KT * MT * 128], BF16)   # col idx (kc*MT+mc)
            whh_T = constp.tile([128, KT * MT * 128], BF16)
            # xp^T, SBUF-resident, col idx (t, hc, b)
            xp_t = constp.tile([128, SL * MT * BL], BF16)
            # initial h^T staging window (holds h0 in slot WIN-1)
            stag_init = constp.tile([128, WIN * MT * BL], BF16)

            with (
                tc.tile_pool(name="wld", bufs=2) as wldp,
                tc.tile_pool(name="wtp", bufs=2, space="PSUM") as wtpp,
            ):
                for w_d, w_T in ((wih_d, wih_T), (whh_d, whh_T)):
                    for mc in range(MT):
                        wld = wldp.tile([128, 512], F32, tag="wld")
                        wldb = wldp.tile([128, 512], BF16, tag="wldb")
                        nc.sync.dma_start(wld[:], w_d[mc * 128 : (mc + 1) * 128, :])
                        nc.vector.tensor_copy(wldb[:], wld[:])
                        for kc in range(KT):
                            ps = wtpp.tile([128, 128], BF16, tag="wtp")
                            nc.tensor.transpose(
                                ps[:], wldb[:, kc * 128 : (kc + 1) * 128], ident[:]
                            )
                            j = (kc * MT + mc) * 128
                            nc.vector.tensor_copy(w_T[:, j : j + 128], ps[:])

                # h0 -> bf16, transposed into stag_init slot WIN-1
                h0l = wldp.tile([BL, H], F32, tag="h0l")
                h0b = wldp.tile([BL, H], BF16, tag="h0b")
                nc.sync.dma_start(h0l[:], h0_d[:, :])
                nc.vector.tensor_copy(h0b[:], h0l[:])
                for hc in range(MT):
                    ps = wtpp.tile([128, 128], BF16, tag="wtp")
                    nc.tensor.transpose(
                        ps[:, :BL], h0b[:, hc * 128 : (hc + 1) * 128],
                        ident[:BL, :BL],
                    )
                    off = hc * WIN * BL + (WIN - 1) * BL
                    nc.vector.tensor_copy(
                        stag_init[:, off : off + BL], ps[:, :BL]
                    )

            # view of xp^T as [p, t, hc, b]
            xp_v = xp_t[:].rearrange("p (t hc b) -> p t hc b", hc=MT, b=BL)

            # ========== phase 1: xp^T = W_ih @ x^T + bias ==========
            TC = 512  # timesteps per block (one batch row at a time)
            with (
                tc.tile_pool(name="xld", bufs=3) as xldp,
                tc.tile_pool(name="xtr", bufs=2) as xtrp,
                tc.tile_pool(name="xps", bufs=4, space="PSUM") as xpsp,
                tc.tile_pool(name="mmp", bufs=2, space="PSUM") as mmpp,
            ):
                for b in range(BL):
                    for tc0 in range(0, SL, TC):
                        ntc = min(TC, SL - tc0)
                        xT = [
                            xtrp.tile(
                                [128, TC], BF16, tag=f"xT{ic}", name=f"xT{ic}"
                            )
                            for ic in range(KT)
                        ]
                        for tb in range((ntc + 127) // 128):
                            nb = min(128, ntc - tb * 128)
                            xld = xldp.tile([128, 512], F32, tag="xld")
                            xlb = xldp.tile([128, 512], BF16, tag="xlb")
                            r0 = tc0 + tb * 128
                            nc.sync.dma_start(
                                xld[:nb, :], x_d[b, r0 : r0 + nb, :]
                            )
                            nc.vector.tensor_copy(xlb[:nb, :], xld[:nb, :])
                            for ic in range(KT):
                                ps = xpsp.tile([128, 128], BF16, tag="xtp")
                                nc.tensor.transpose(
                                    ps[:, :nb],
                                    xlb[:nb, ic * 128 : (ic + 1) * 128],
                                    ident[:nb, :nb],
                                )
                                nc.vector.tensor_copy(
                                    xT[ic][:, tb * 128 : tb * 128 + nb],
                                    ps[:, :nb],
                                )
                        for hc in range(MT):
                            ps = mmpp.tile([128, TC], F32, tag="xpmm")
                            for ic in range(KT):
                                j = (ic * MT + hc) * 128
                                nc.tensor.matmul(
                                    ps[:, :ntc],
                                    wih_T[:, j : j + 128],
                                    xT[ic][:, :ntc],
                                    start=(ic == 0),
                                    stop=(ic == KT - 1),
                                )
                            # xp_v[:, tc0:tc0+ntc, hc, b] += bias
                            nc.scalar.add(
                                xp_v[:, tc0 : tc0 + ntc, hc, b],
                                ps[:, :ntc],
                                bias_t[:, hc : hc + 1],
                            )

            # ========== phase 2: the recurrence ==========
            # Per step: 2 identity matmuls inject xp_t into PSUM, then 16
            # W_hh matmuls accumulate; relu is a single DVE max per
            # half-step (two "super-groups" of h-chunks {0,1} / {2,3}) so
            # consecutive steps pipeline on the PE without stalls.
            with (
                tc.tile_pool(name="rec", bufs=2, space="PSUM") as recp,
                tc.tile_pool(name="stg", bufs=3) as stgp,
                tc.tile_pool(name="otr", bufs=2, space="PSUM") as otrp,
                tc.tile_pool(name="owd", bufs=3) as owdp,
            ):
                def finalize_window(w, stag_w):
                    """Transpose window w to batch-major and DMA it out."""
                    ow = owdp.tile([128, H], F32, tag="ow")
                    for hc in range(MT):
                        ps = otrp.tile([128, 128], BF16, tag="otr")
                        src_ap = stag_w[:, hc * WIN * BL : (hc + 1) * WIN * BL]
                        nc.tensor.transpose(ps[:], src_ap, ident[:])
                        nc.vector.tensor_copy(
                            ow[:, hc * 128 : (hc + 1) * 128], ps[:]
                        )
                    dst = out_d[:, w * WIN : (w + 1) * WIN, :].rearrange(
                        "b t h -> t b h"
                    )
                    nc.sync.dma_start(dst, ow[:])
                    return ow

                stag_prev = stag_init
                stag_cur = None
                for t in range(SL):
                    tl = t % WIN
                    if tl == 0:
                        if t > 0:
                            finalize_window(t // WIN - 1, stag_prev)
                        stag_cur = stgp.tile([128, WIN * MT * BL], BF16, tag="stg")
                    ps = recp.tile([128, MT * BL], F32, tag="rec")
                    ptl = (WIN - 1) if tl == 0 else (tl - 1)
                    hprev = stag_prev if tl == 0 else stag_cur
                    sc = stag_cur

                    def mm_id(g):
                        # inject xp_t[:, t, 2g:2g+2, :] via identity matmul
                        o = t * MT * BL + g * 2 * BL
                        nc.tensor.matmul(
                            ps[:, g * 2 * BL : (g + 1) * 2 * BL],
                            ident[:],
                            xp_t[:, o : o + 2 * BL],
                            start=(g == 0),
                            stop=False,
                            skip_group_check=True,
                        )

                    def mm(k, m):
                        j = (k * MT + m) * 128
                        o = k * WIN * BL + ptl * BL
                        nc.tensor.matmul(
                            ps[:, m * BL : (m + 1) * BL],
                            whh_T[:, j : j + 128],
                            hprev[:, o : o + BL],
                            start=False,
                            stop=(k == KT - 1),
                            skip_group_check=True,
                        )

                    def epilogue(g):
                        # relu: stag[hc=2g:2g+2, tl, :] = max(psum, 0), bf16
                        dst = sc[:].rearrange(
                            "p (hc t b) -> p hc t b", t=WIN, b=BL
                        )[:, 2 * g : 2 * g + 2, tl, :]
                        nc.vector.tensor_scalar_max(
                            dst, ps[:, g * 2 * BL : (g + 1) * 2 * BL], 0.0
                        )

                    mm_id(0)
                    mm_id(1)
                    if t % 2 == 0:  # m-outer: super-groups finish early/late
                        for m in range(MT):
                            for k in range(KT):
                                mm(k, m)
                            if m % 2 == 1:
                                epilogue(m // 2)
                    else:  # k-outer: consumes h chunks in k order
                        for k in range(KT):
                            for m in range(MT):
                                mm(k, m)
                                if k == KT - 1 and m % 2 == 1:
                                    epilogue(m // 2)
                    if tl == WIN - 1:
                        stag_prev = stag_cur

                last_ow = finalize_window(NW - 1, stag_prev)
                # h_final: rows t=WIN-1 of the last window = partitions 120..127
                nc.sync.dma_start(
                    hf_d[:, :], last_ow[(WIN - 1) * BL : WIN * BL, :]
                )

    _split_multi_waits(nc)
    return nc


_NC_CACHE = {}


def get_nc(seq_len=S):
    if seq_len not in _NC_CACHE:
        _NC_CACHE[seq_len] = build_rnn_kernel(seq_len)
    return _NC_CACHE[seq_len]


def make_in_maps(inputs, h0, weight_ih, weight_hh, bias_ih, bias_hh):
    inputs = np.ascontiguousarray(np.asarray(inputs, dtype=np.float32))
    h0 = np.asarray(h0, dtype=np.float32).reshape(-1, H)
    weight_ih = np.ascontiguousarray(np.asarray(weight_ih, dtype=np.float32))
    weight_hh = np.ascontiguousarray(np.asarray(weight_hh, dtype=np.float32))
    bias_ih = np.ascontiguousarray(np.asarray(bias_ih, dtype=np.float32))
    bias_hh = np.ascontiguousarray(np.asarray(bias_hh, dtype=np.float32))
    in_maps = []
    for c in range(NCORES):
        sl = slice(c * BL, (c + 1) * BL)
        in_maps.append(
            {
                "inputs": np.ascontiguousarray(inputs[sl]),
                "h0": np.ascontiguousarray(h0[sl]),
                "weight_ih": weight_ih,
                "weight_hh": weight_hh,
                "bias_ih": bias_ih,
                "bias_hh": bias_hh,
            }
        )
    return in_maps


def assemble(results, seq_len=S):
    outputs = np.empty((B, seq_len, H), dtype=np.float32)
    h_final = np.empty((B, H), dtype=np.float32)
    for c in range(NCORES):
        sl = slice(c * BL, (c + 1) * BL)
        outputs[sl] = results[c]["outputs"]
        h_final[sl] = results[c]["h_final"]
    return outputs, h_final[None, :, :]


def kernel(inputs, h0, weight_ih, weight_hh, bias_ih, bias_hh):
    nc = get_nc(S)
    in_maps = make_in_maps(inputs, h0, weight_ih, weight_hh, bias_ih, bias_hh)
    res = run_bass_kernel_spmd(nc, in_maps, core_ids=list(range(NCORES)))
    return assemble(res.results, S)


# revision 11
# speedup vs baseline: 1.3279x; 1.0166x over previous
"""Trainium2 Bass kernel for the ReLU-RNN problem (nn_RNN).

Math (per core, data-parallel over batch; 8 batch rows per core):
    xp  = x @ W_ih.T + b_ih + b_hh          (big GEMM, phase 1)
    h_t = relu(xp_t + h_{t-1} @ W_hh.T)     (S sequential steps, phase 2)

All recurrent state lives transposed: h^T is [H=512 (4 partition
chunks), B_local=8].  Each step is 16 bf16 matmuls (stationary =
128x128 tiles of W_hh^T, moving = h^T chunks [128, 8]) accumulated in
fp32 PSUM, then add-xp + relu split per output h-chunk so consecutive
steps pipeline on the PE without stalls (m-outer / k-outer orderings
alternate between steps).  Every 16 steps the h^T history window is
PE-transposed back to batch-major and DMAed out as [b, t, :] rows.
"""

import numpy as np

import concourse.bass as bass
import concourse.mybir as mybir
from concourse import masks, tile
from concourse.bass_utils import run_bass_kernel_spmd

B, S, I, H = 64, 1024, 512, 512
NCORES = 8
BL = B // NCORES          # batch rows per core
KT = I // 128             # contraction tiles (4)
MT = H // 128             # output-row tiles (4)
WIN = 16                  # recurrence steps per output window
F32 = mybir.dt.float32
BF16 = mybir.dt.bfloat16

# This walrus build rejects instructions carrying more than one sync-wait
# command.  After Tile scheduling, split any excess waits onto same-engine
# NOP instructions inserted immediately before the offending instruction.
_MAX_WAITS = 1


def _split_multi_waits(nc):
    cnt = 0
    for bb in nc.main_func.blocks:
        insts = bb.instructions
        i = 0
        while i < len(insts):
            inst = insts[i]
            si = getattr(inst, "sync_info", None)
            if si is not None and len(si.on_wait) > _MAX_WAITS:
                waits = list(si.on_wait)
                si.on_wait = waits[-_MAX_WAITS:]
                rest = waits[:-_MAX_WAITS]
                for j in range(0, len(rest), _MAX_WAITS):
                    nop = mybir.InstNoOp(
                        name=f"ws_{cnt}",
                        engine=inst.engine,
                        sync_info=mybir.SyncInfo(
                            on_wait=rest[j : j + _MAX_WAITS], on_update=[]
                        ),
                        bass_nofuse=True,
                    )
                    cnt += 1
                    insts.insert(i, nop)
                    i += 1
            i += 1
    return cnt


def build_rnn_kernel(seq_len=S):
    SL = seq_len
    NW = SL // WIN
    assert SL % WIN == 0
    nc = bass.Bass(target_bir_lowering=False, trn_type="TRN2")

    x_d = nc.dram_tensor("inputs", [BL, SL, I], F32, kind="ExternalInput")
    h0_d = nc.dram_tensor("h0", [BL, H], F32, kind="ExternalInput")
    wih_d = nc.dram_tensor("weight_ih", [H, I], F32, kind="ExternalInput")
    whh_d = nc.dram_tensor("weight_hh", [H, H], F32, kind="ExternalInput")
    bih_d = nc.dram_tensor("bias_ih", [H], F32, kind="ExternalInput")
    bhh_d = nc.dram_tensor("bias_hh", [H], F32, kind="ExternalInput")
    out_d = nc.dram_tensor("outputs", [BL, SL, H], F32, kind="ExternalOutput")
    hf_d = nc.dram_tensor("h_final", [BL, H], F32, kind="ExternalOutput")

    with tile.TileContext(nc) as tc:
        with tc.tile_pool(name="const", bufs=1) as constp:
            ident = constp.tile([128, 128], BF16)
            masks.make_identity(nc, ident[:])

            # bias_t[p, hc] = (b_ih + b_hh)[hc*128 + p]
            bias_t = constp.tile([128, MT], F32)
            btmp = constp.tile([128, MT], F32)
            nc.sync.dma_start(bias_t[:], bih_d.rearrange("(hc p) -> p hc", p=128))
            nc.sync.dma_start(btmp[:], bhh_d.rearrange("(hc p) -> p hc", p=128))
            nc.vector.tensor_add(bias_t[:], bias_t[:], btmp[:])

            # lhsT weight tiles: lhsT(kc, mc)[k, m] = W[mc*128+m, kc*128+k]
            wih_T = constp.tile([128, KT * MT * 128], BF16)   # col idx (kc*MT+mc)
            whh_T = constp.tile([128, KT * MT * 128], BF16)
            # xp^T, SBUF-resident, col idx (t, hc, b)
            xp_t = constp.tile([128, SL * MT * BL], BF16)
            # initial h^T staging window (holds h0 in slot WIN-1)
            stag_init = constp.tile([128, WIN * MT * BL], BF16)

            with (
                tc.tile_pool(name="wld", bufs=2) as wldp,
                tc.tile_pool(name="wtp", bufs=2, space="PSUM") as wtpp,
            ):
                for w_d, w_T in ((wih_d, wih_T), (whh_d, whh_T)):
                    for mc in range(MT):
                        wld = wldp.tile([128, 512], F32, tag="wld")
                        wldb = wldp.tile([128, 512], BF16, tag="wldb")
                        nc.sync.dma_start(wld[:], w_d[mc * 128 : (mc + 1) * 128, :])
                        nc.vector.tensor_copy(wldb[:], wld[:])
                        for kc in range(KT):
                            ps = wtpp.tile([128, 128], BF16, tag="wtp")
                            nc.tensor.transpose(
                                ps[:], wldb[:, kc * 128 : (kc + 1) * 128], ident[:]
                            )
                            j = (kc * MT + mc) * 128
                            nc.vector.tensor_copy(w_T[:, j : j + 128], ps[:])

                # h0 -> bf16, transposed into stag_init slot WIN-1
                h0l = wldp.tile([BL, H], F32, tag="h0l")
                h0b = wldp.tile([BL, H], BF16, tag="h0b")
                nc.sync.dma_start(h0l[:], h0_d[:, :])
                nc.vector.tensor_copy(h0b[:], h0l[:])
                for hc in range(MT):
                    ps = wtpp.tile([128, 128], BF16, tag="wtp")
                    nc.tensor.transpose(
                        ps[:, :BL], h0b[:, hc * 128 : (hc + 1) * 128],
                        ident[:BL, :BL],
                    )
                    off = hc * WIN * BL + (WIN - 1) * BL
                    nc.vector.tensor_copy(
                        stag_init[:, off : off + BL], ps[:, :BL]
                    )

            # view of xp^T as [p, t, hc, b]
            xp_v = xp_t[:].rearrange("p (t hc b) -> p t hc b", hc=MT, b=BL)

            # ========== phase 1: xp^T = W_ih @ x^T + bias ==========
            TC = 512  # timesteps per block (one batch row at a time)
            with (
                tc.tile_pool(name="xld", bufs=3) as xldp,
                tc.tile_pool(name="xtr", bufs=2) as xtrp,
                tc.tile_pool(name="xps", bufs=4, space="PSUM") as xpsp,
                tc.tile_pool(name="mmp", bufs=2, space="PSUM") as mmpp,
            ):
                for b in range(BL):
                    for tc0 in range(0, SL, TC):
                        ntc = min(TC, SL - tc0)
                        xT = [
                            xtrp.tile(
                                [128, TC], BF16, tag=f"xT{ic}", name=f"xT{ic}"
                            )
                            for ic in range(KT)
                        ]
                        for tb in range((ntc + 127) // 128):
                            nb = min(128, ntc - tb * 128)
                            xld = xldp.tile([128, 512], F32, tag="xld")
                            xlb = xldp.tile([128, 512], BF16, tag="xlb")
                            r0 = tc0 + tb * 128
                            nc.sync.dma_start(
                                xld[:nb, :], x_d[b, r0 : r0 + nb, :]
                            )
                            nc.vector.tensor_copy(xlb[:nb, :], xld[:nb, :])
                            for ic in range(KT):
                                ps = xpsp.tile([128, 128], BF16, tag="xtp")
                                nc.tensor.transpose(
                                    ps[:, :nb],
                                    xlb[:nb, ic * 128 : (ic + 1) * 128],
                                    ident[:nb, :nb],
                                )
                                nc.vector.tensor_copy(
                                    xT[ic][:, tb * 128 : tb * 128 + nb],
                                    ps[:, :nb],
                                )
                        for hc in range(MT):
                            ps = mmpp.tile([128, TC], F32, tag="xpmm")
                            for ic in range(KT):
                                j = (ic * MT + hc) * 128
                                nc.tensor.matmul(
                                    ps[:, :ntc],
                                    wih_T[:, j : j + 128],
                                    xT[ic][:, :ntc],
                                    start=(ic == 0),
                                    stop=(ic == KT - 1),
                                )
                            # xp_v[:, tc0:tc0+ntc, hc, b] += bias
                            nc.scalar.add(
                                xp_v[:, tc0 : tc0 + ntc, hc, b],
                                ps[:, :ntc],
                                bias_t[:, hc : hc + 1],
                            )

            # ========== phase 2: the recurrence ==========
            # Per step: 2 identity matmuls inject xp_t into PSUM, then 16
            # W_hh matmuls accumulate; relu is a single DVE max per
            # half-step (two "super-groups" of h-chunks {0,1} / {2,3}) so
            # consecutive steps pipeline on the PE without stalls.
            with (
                tc.tile_pool(name="rec", bufs=2, space="PSUM") as recp,
                tc.tile_pool(name="stg", bufs=3) as stgp,
                tc.tile_pool(name="otr", bufs=2, space="PSUM") as otrp,
                tc.tile_pool(name="owd", bufs=3) as owdp,
            ):
                def finalize_window(w, stag_w):
                    """Transpose window w to batch-major and DMA it out."""
                    ow = owdp.tile([128, H], F32, tag="ow")
                    for hc in range(MT):
                        ps = otrp.tile([128, 128], BF16, tag="otr")
                        src_ap = stag_w[:, hc * WIN * BL : (hc + 1) * WIN * BL]
                        nc.tensor.transpose(ps[:], src_ap, ident[:])
                        nc.vector.tensor_copy(
                            ow[:, hc * 128 : (hc + 1) * 128], ps[:]
                        )
                    dst = out_d[:, w * WIN : (w + 1) * WIN, :].rearrange(
                        "b t h -> t b h"
                    )
                    nc.sync.dma_start(dst, ow[:])
                    return ow

                stag_prev = stag_init
                stag_cur = None
                ow = None
                for t in range(SL):
                    tl = t % WIN
                    if tl == 0:
                        stag_fin = stag_prev if t > 0 else None
                        stag_cur = stgp.tile([128, WIN * MT * BL], BF16, tag="stg")
                        if t > 0:
                            ow = owdp.tile([128, H], F32, tag="ow", name="ow")
                    ps = recp.tile([128, MT * BL], F32, tag="rec")
                    ptl = (WIN - 1) if tl == 0 else (tl - 1)
                    hprev = stag_prev if tl == 0 else stag_cur
                    sc = stag_cur

                    def mm_id(g, first):
                        # inject xp_t[:, t, 2g:2g+2, :] via identity matmul
                        o = t * MT * BL + g * 2 * BL
                        nc.tensor.matmul(
                            ps[:, g * 2 * BL : (g + 1) * 2 * BL],
                            ident[:],
                            xp_t[:, o : o + 2 * BL],
                            start=first,
                            stop=False,
                            skip_group_check=True,
                        )

                    def mm(k, m):
                        j = (k * MT + m) * 128
                        o = k * WIN * BL + ptl * BL
                        nc.tensor.matmul(
                            ps[:, m * BL : (m + 1) * BL],
                            whh_T[:, j : j + 128],
                            hprev[:, o : o + BL],
                            start=False,
                            stop=(k == KT - 1),
                            skip_group_check=True,
                        )

                    def epilogue(g):
                        # relu: stag[hc=2g:2g+2, tl, :] = max(psum, 0), bf16
                        dst = sc[:].rearrange(
                            "p (hc t b) -> p hc t b", t=WIN, b=BL
                        )[:, 2 * g : 2 * g + 2, tl, :]
                        nc.vector.tensor_scalar_max(
                            dst, ps[:, g * 2 * BL : (g + 1) * 2 * BL], 0.0
                        )

                    # Uniform schedule: sg0 (chunks 0,1) produced in the first
                    # 8 matmuls so its relu lands ~10 slots before step t+1
                    # consumes it; k2/k3 consumption is pushed late because
                    # sg1(t-1) finishes at the end of step t-1.
                    mm_id(0, first=True)
                    if t >= WIN and tl < MT:
                        # fill the k2-wait with one window-finalize transpose
                        hc = tl
                        w = t // WIN - 1
                        pst = otrp.tile([128, 128], BF16, tag="otr", name="otr")
                        nc.tensor.transpose(
                            pst[:],
                            stag_fin[:, hc * WIN * BL : (hc + 1) * WIN * BL],
                            ident[:],
                        )
                        nc.vector.tensor_copy(
                            ow[:, hc * 128 : (hc + 1) * 128], pst[:]
                        )
                    for k in (0, 1, 2, 3):
                        mm(k, 0)
                        mm(k, 1)
                    epilogue(0)
                    mm_id(1, first=False)
                    for k in (0, 1, 2, 3):
                        mm(k, 2)
                        mm(k, 3)
                    epilogue(1)
                    if t >= WIN and tl == MT:
                        w = t // WIN - 1
                        dst = out_d[:, w * WIN : (w + 1) * WIN, :].rearrange(
                            "b t h -> t b h"
                        )
                        nc.sync.dma_start(dst, ow[:])
                    if tl == WIN - 1:
                        stag_prev = stag_cur

                last_ow = finalize_window(NW - 1, stag_prev)
                # h_final: rows t=WIN-1 of the last window = partitions 120..127
                nc.sync.dma_start(
                    hf_d[:, :], last_ow[(WIN - 1) * BL : WIN * BL, :]
                )

    _split_multi_waits(nc)
    return nc


_NC_CACHE = {}


def get_nc(seq_len=S):
    if seq_len not in _NC_CACHE:
        _NC_CACHE[seq_len] = build_rnn_kernel(seq_len)
    return _NC_CACHE[seq_len]


def make_in_maps(inputs, h0, weight_ih, weight_hh, bias_ih, bias_hh):
    inputs = np.ascontiguousarray(np.asarray(inputs, dtype=np.float32))
    h0 = np.asarray(h0, dtype=np.float32).reshape(-1, H)
    weight_ih = np.ascontiguousarray(np.asarray(weight_ih, dtype=np.float32))
    weight_hh = np.ascontiguousarray(np.asarray(weight_hh, dtype=np.float32))
    bias_ih = np.ascontiguousarray(np.asarray(bias_ih, dtype=np.float32))
    bias_hh = np.ascontiguousarray(np.asarray(bias_hh, dtype=np.float32))
    in_maps = []
    for c in range(NCORES):
        sl = slice(c * BL, (c + 1) * BL)
        in_maps.append(
            {
                "inputs": np.ascontiguousarray(inputs[sl]),
                "h0": np.ascontiguousarray(h0[sl]),
                "weight_ih": weight_ih,
                "weight_hh": weight_hh,
                "bias_ih": bias_ih,
                "bias_hh": bias_hh,
            }
        )
    return in_maps


def assemble(results, seq_len=S):
    outputs = np.empty((B, seq_len, H), dtype=np.float32)
    h_final = np.empty((B, H), dtype=np.float32)
    for c in range(NCORES):
        sl = slice(c * BL, (c + 1) * BL)
        outputs[sl] = results[c]["outputs"]
        h_final[sl] = results[c]["h_final"]
    return outputs, h_final[None, :, :]


def kernel(inputs, h0, weight_ih, weight_hh, bias_ih, bias_hh):
    nc = get_nc(S)
    in_maps = make_in_maps(inputs, h0, weight_ih, weight_hh, bias_ih, bias_hh)
    res = run_bass_kernel_spmd(nc, in_maps, core_ids=list(range(NCORES)))
    return assemble(res.results, S)


# revision 12
# speedup vs baseline: 1.6819x; 1.2666x over previous
"""Trainium2 Bass kernel for the ReLU-RNN problem (nn_RNN).

Math (per core, data-parallel over batch; 8 batch rows per core):
    xp  = x @ W_ih.T + b_ih + b_hh          (big GEMM, phase 1)
    h_t = relu(xp_t + h_{t-1} @ W_hh.T)     (S sequential steps, phase 2)

All recurrent state lives transposed: h^T is [H=512 (4 partition
chunks), B_local=8].  Each step is 16 bf16 matmuls (stationary =
128x128 tiles of W_hh^T, moving = h^T chunks [128, 8]) accumulated in
fp32 PSUM, then add-xp + relu split per output h-chunk so consecutive
steps pipeline on the PE without stalls (m-outer / k-outer orderings
alternate between steps).  Every 16 steps the h^T history window is
PE-transposed back to batch-major and DMAed out as [b, t, :] rows.
"""

import numpy as np

import concourse.bass as bass
import concourse.mybir as mybir
from concourse import masks, tile
from concourse.bass_utils import run_bass_kernel_spmd

B, S, I, H = 64, 1024, 512, 512
NCORES = 8
BL = B // NCORES          # batch rows per core
KT = I // 128             # contraction tiles (4)
MT = H // 128             # output-row tiles (4)
WIN = 16                  # recurrence steps per output window
F32 = mybir.dt.float32
BF16 = mybir.dt.bfloat16

# This walrus build rejects instructions carrying more than one sync-wait
# command.  After Tile scheduling, split any excess waits onto same-engine
# NOP instructions inserted immediately before the offending instruction.
_MAX_WAITS = 1


def _split_multi_waits(nc):
    cnt = 0
    for bb in nc.main_func.blocks:
        insts = bb.instructions
        i = 0
        while i < len(insts):
            inst = insts[i]
            si = getattr(inst, "sync_info", None)
            if si is not None and len(si.on_wait) > _MAX_WAITS:
                waits = list(si.on_wait)
                si.on_wait = waits[-_MAX_WAITS:]
                rest = waits[:-_MAX_WAITS]
                for j in range(0, len(rest), _MAX_WAITS):
                    nop = mybir.InstNoOp(
                        name=f"ws_{cnt}",
                        engine=inst.engine,
                        sync_info=mybir.SyncInfo(
                            on_wait=rest[j : j + _MAX_WAITS], on_update=[]
                        ),
                        bass_nofuse=True,
                    )
                    cnt += 1
                    insts.insert(i, nop)
                    i += 1
            i += 1
    return cnt


def build_rnn_kernel(seq_len=S):
    SL = seq_len
    NW = SL // WIN
    assert SL % WIN == 0
    nc = bass.Bass(target_bir_lowering=False, trn_type="TRN2")

    x_d = nc.dram_tensor("inputs", [BL, SL, I], F32, kind="ExternalInput")
    h0_d = nc.dram_tensor("h0", [BL, H], F32, kind="ExternalInput")
    wih_d = nc.dram_tensor("weight_ih", [H, I], F32, kind="ExternalInput")
    whh_d = nc.dram_tensor("weight_hh", [H, H], F32, kind="ExternalInput")
    bih_d = nc.dram_tensor("bias_ih", [H], F32, kind="ExternalInput")
    bhh_d = nc.dram_tensor("bias_hh", [H], F32, kind="ExternalInput")
    out_d = nc.dram_tensor("outputs", [BL, SL, H], F32, kind="ExternalOutput")
    hf_d = nc.dram_tensor("h_final", [BL, H], F32, kind="ExternalOutput")

    with tile.TileContext(nc) as tc:
        with tc.tile_pool(name="const", bufs=1) as constp:
            ident = constp.tile([128, 128], BF16)
            masks.make_identity(nc, ident[:])

            # bias_t[p, hc] = (b_ih + b_hh)[hc*128 + p]
            bias_t = constp.tile([128, MT], F32)
            btmp = constp.tile([128, MT], F32)
            nc.sync.dma_start(bias_t[:], bih_d.rearrange("(hc p) -> p hc", p=128))
            nc.sync.dma_start(btmp[:], bhh_d.rearrange("(hc p) -> p hc", p=128))
            nc.vector.tensor_add(bias_t[:], bias_t[:], btmp[:])

            # lhsT weight tiles: lhsT(kc, mc)[k, m] = W[mc*128+m, kc*128+k]
            wih_T = constp.tile([128, KT * MT * 128], BF16)   # col idx (kc*MT+mc)
            whh_T = constp.tile([128, KT * MT * 128], BF16)
            # xp^T, SBUF-resident, col idx (t, hc, b)
            xp_t = constp.tile([128, SL * MT * BL], BF16)
            # initial h^T staging window (holds h0 in slot WIN-1)
            stag_init = constp.tile([128, WIN * MT * BL], BF16)

            with (
                tc.tile_pool(name="wld", bufs=2) as wldp,
                tc.tile_pool(name="wtp", bufs=2, space="PSUM") as wtpp,
            ):
                for w_d, w_T in ((wih_d, wih_T), (whh_d, whh_T)):
                    for mc in range(MT):
                        wld = wldp.tile([128, 512], F32, tag="wld")
                        wldb = wldp.tile([128, 512], BF16, tag="wldb")
                        nc.sync.dma_start(wld[:], w_d[mc * 128 : (mc + 1) * 128, :])
                        nc.vector.tensor_copy(wldb[:], wld[:])
                        for kc in range(KT):
                            ps = wtpp.tile([128, 128], BF16, tag="wtp")
                            nc.tensor.transpose(
                                ps[:], wldb[:, kc * 128 : (kc + 1) * 128], ident[:]
                            )
                            j = (kc * MT + mc) * 128
                            nc.vector.tensor_copy(w_T[:, j : j + 128], ps[:])

                # h0 -> bf16, transposed into stag_init slot WIN-1
                h0l = wldp.tile([BL, H], F32, tag="h0l")
                h0b = wldp.tile([BL, H], BF16, tag="h0b")
                nc.sync.dma_start(h0l[:], h0_d[:, :])
                nc.vector.tensor_copy(h0b[:], h0l[:])
                for hc in range(MT):
                    ps = wtpp.tile([128, 128], BF16, tag="wtp")
                    nc.tensor.transpose(
                        ps[:, :BL], h0b[:, hc * 128 : (hc + 1) * 128],
                        ident[:BL, :BL],
                    )
                    off = hc * WIN * BL + (WIN - 1) * BL
                    nc.vector.tensor_copy(
                        stag_init[:, off : off + BL], ps[:, :BL]
                    )

            # view of xp^T as [p, t, hc, b]
            xp_v = xp_t[:].rearrange("p (t hc b) -> p t hc b", hc=MT, b=BL)

            # ========== phase 1: xp^T = W_ih @ x^T + bias ==========
            TC = 512  # timesteps per block (one batch row at a time)
            with (
                tc.tile_pool(name="xld", bufs=3) as xldp,
                tc.tile_pool(name="xtr", bufs=2) as xtrp,
                tc.tile_pool(name="xps", bufs=4, space="PSUM") as xpsp,
                tc.tile_pool(name="mmp", bufs=2, space="PSUM") as mmpp,
            ):
                for b in range(BL):
                    for tc0 in range(0, SL, TC):
                        ntc = min(TC, SL - tc0)
                        xT = [
                            xtrp.tile(
                                [128, TC], BF16, tag=f"xT{ic}", name=f"xT{ic}"
                            )
                            for ic in range(KT)
                        ]
                        for tb in range((ntc + 127) // 128):
                            nb = min(128, ntc - tb * 128)
                            xld = xldp.tile([128, 512], F32, tag="xld")
                            xlb = xldp.tile([128, 512], BF16, tag="xlb")
                            r0 = tc0 + tb * 128
                            nc.sync.dma_start(
                                xld[:nb, :], x_d[b, r0 : r0 + nb, :]
                            )
                            nc.vector.tensor_copy(xlb[:nb, :], xld[:nb, :])
                            for ic in range(KT):
                                ps = xpsp.tile([128, 128], BF16, tag="xtp")
                                nc.tensor.transpose(
                                    ps[:, :nb],
                                    xlb[:nb, ic * 128 : (ic + 1) * 128],
                                    ident[:nb, :nb],
                                )
                                nc.vector.tensor_copy(
                                    xT[ic][:, tb * 128 : tb * 128 + nb],
                                    ps[:, :nb],
                                )
                        for hc in range(MT):
                            ps = mmpp.tile([128, TC], F32, tag="xpmm")
                            for ic in range(KT):
                                j = (ic * MT + hc) * 128
                                nc.tensor.matmul(
                                    ps[:, :ntc],
                                    wih_T[:, j : j + 128],
                                    xT[ic][:, :ntc],
                                    start=(ic == 0),
                                    stop=(ic == KT - 1),
                                )
                            # xp_v[:, tc0:tc0+ntc, hc, b] += bias
                            nc.scalar.add(
                                xp_v[:, tc0 : tc0 + ntc, hc, b],
                                ps[:, :ntc],
                                bias_t[:, hc : hc + 1],
                            )

            # ========== phase 2: the recurrence ==========
            # Per step: 2 identity matmuls inject xp_t into PSUM, then 16
            # W_hh matmuls accumulate; relu is a single DVE max per
            # half-step (two "super-groups" of h-chunks {0,1} / {2,3}) so
            # consecutive steps pipeline on the PE without stalls.
            with (
                tc.tile_pool(name="rec", bufs=2, space="PSUM") as recp,
                tc.tile_pool(name="stg", bufs=3) as stgp,
                tc.tile_pool(name="otr", bufs=2, space="PSUM") as otrp,
                tc.tile_pool(name="owd", bufs=3) as owdp,
            ):
                def finalize_window(w, stag_w):
                    """Transpose window w to batch-major and DMA it out."""
                    ow = owdp.tile([128, H], F32, tag="ow")
                    for hc in range(MT):
                        ps = otrp.tile([128, 128], BF16, tag="otr")
                        src_ap = stag_w[:, hc * WIN * BL : (hc + 1) * WIN * BL]
                        nc.tensor.transpose(ps[:], src_ap, ident[:])
                        nc.vector.tensor_copy(
                            ow[:, hc * 128 : (hc + 1) * 128], ps[:]
                        )
                    dst = out_d[:, w * WIN : (w + 1) * WIN, :].rearrange(
                        "b t h -> t b h"
                    )
                    nc.sync.dma_start(dst, ow[:])
                    return ow

                stag_prev = stag_init
                stag_cur = None
                ow = None
                for t in range(SL):
                    tl = t % WIN
                    if tl == 0:
                        stag_fin = stag_prev if t > 0 else None
                        stag_cur = stgp.tile([128, WIN * MT * BL], BF16, tag="stg")
                        if t > 0:
                            ow = owdp.tile([128, H], F32, tag="ow", name="ow")
                    # separate PSUM banks per half-step so the DVE relu of
                    # one half never blocks PE writes of the other
                    psA = recp.tile([128, 2 * BL], F32, tag="recA", name="psA")
                    psB = recp.tile([128, 2 * BL], F32, tag="recB", name="psB")
                    psx = (psA, psB)
                    ptl = (WIN - 1) if tl == 0 else (tl - 1)
                    hprev = stag_prev if tl == 0 else stag_cur
                    sc = stag_cur

                    def mm_id(g):
                        # inject xp_t[:, t, 2g:2g+2, :] via identity matmul
                        o = t * MT * BL + g * 2 * BL
                        nc.tensor.matmul(
                            psx[g][:],
                            ident[:],
                            xp_t[:, o : o + 2 * BL],
                            start=True,
                            stop=False,
                            skip_group_check=True,
                        )

                    def mm(k, m):
                        j = (k * MT + m) * 128
                        o = k * WIN * BL + ptl * BL
                        nc.tensor.matmul(
                            psx[m // 2][:, (m % 2) * BL : (m % 2 + 1) * BL],
                            whh_T[:, j : j + 128],
                            hprev[:, o : o + BL],
                            start=False,
                            stop=(k == KT - 1),
                            skip_group_check=True,
                        )

                    def epilogue(g):
                        # relu: stag[hc=2g:2g+2, tl, :] = max(psum, 0), bf16
                        dst = sc[:].rearrange(
                            "p (hc t b) -> p hc t b", t=WIN, b=BL
                        )[:, 2 * g : 2 * g + 2, tl, :]
                        nc.vector.tensor_scalar_max(dst, psx[g][:], 0.0)

                    # Uniform schedule: sg0 (chunks 0,1) produced in the first
                    # 8 matmuls so its relu lands ~10 slots before step t+1
                    # consumes it; k2/k3 consumption is pushed late because
                    # sg1(t-1) finishes at the end of step t-1.
                    mm_id(0)
                    if t >= WIN and tl < MT:
                        # fill the k2-wait with one window-finalize transpose
                        hc = tl
                        w = t // WIN - 1
                        pst = otrp.tile([128, 128], BF16, tag="otr", name="otr")
                        nc.tensor.transpose(
                            pst[:],
                            stag_fin[:, hc * WIN * BL : (hc + 1) * WIN * BL],
                            ident[:],
                        )
                        nc.vector.tensor_copy(
                            ow[:, hc * 128 : (hc + 1) * 128], pst[:]
                        )
                    for k in (0, 1, 2, 3):
                        mm(k, 0)
                        mm(k, 1)
                    epilogue(0)
                    mm_id(1)
                    for k in (0, 1, 2, 3):
                        mm(k, 2)
                        mm(k, 3)
                    epilogue(1)
                    if t >= WIN and tl == MT:
                        w = t // WIN - 1
                        dst = out_d[:, w * WIN : (w + 1) * WIN, :].rearrange(
                            "b t h -> t b h"
                        )
                        nc.sync.dma_start(dst, ow[:])
                    if tl == WIN - 1:
                        stag_prev = stag_cur

                last_ow = finalize_window(NW - 1, stag_prev)
                # h_final: rows t=WIN-1 of the last window = partitions 120..127
                nc.sync.dma_start(
                    hf_d[:, :], last_ow[(WIN - 1) * BL : WIN * BL, :]
                )

    _split_multi_waits(nc)
    return nc


_NC_CACHE = {}


def get_nc(seq_len=S):
    if seq_len not in _NC_CACHE:
        _NC_CACHE[seq_len] = build_rnn_kernel(seq_len)
    return _NC_CACHE[seq_len]


def make_in_maps(inputs, h0, weight_ih, weight_hh, bias_ih, bias_hh):
    inputs = np.ascontiguousarray(np.asarray(inputs, dtype=np.float32))
    h0 = np.asarray(h0, dtype=np.float32).reshape(-1, H)
    weight_ih = np.ascontiguousarray(np.asarray(weight_ih, dtype=np.float32))
    weight_hh = np.ascontiguousarray(np.asarray(weight_hh, dtype=np.float32))
    bias_ih = np.ascontiguousarray(np.asarray(bias_ih, dtype=np.float32))
    bias_hh = np.ascontiguousarray(np.asarray(bias_hh, dtype=np.float32))
    in_maps = []
    for c in range(NCORES):
        sl = slice(c * BL, (c + 1) * BL)
        in_maps.append(
            {
                "inputs": np.ascontiguousarray(inputs[sl]),
                "h0": np.ascontiguousarray(h0[sl]),
                "weight_ih": weight_ih,
                "weight_hh": weight_hh,
                "bias_ih": bias_ih,
                "bias_hh": bias_hh,
            }
        )
    return in_maps


def assemble(results, seq_len=S):
    outputs = np.empty((B, seq_len, H), dtype=np.float32)
    h_final = np.empty((B, H), dtype=np.float32)
    for c in range(NCORES):
        sl = slice(c * BL, (c + 1) * BL)
        outputs[sl] = results[c]["outputs"]
        h_final[sl] = results[c]["h_final"]
    return outputs, h_final[None, :, :]


def kernel(inputs, h0, weight_ih, weight_hh, bias_ih, bias_hh):
    nc = get_nc(S)
    in_maps = make_in_maps(inputs, h0, weight_ih, weight_hh, bias_ih, bias_hh)
    res = run_bass_kernel_spmd(nc, in_maps, core_ids=list(range(NCORES)))
    return assemble(res.results, S)
